# revision 34
# baseline (speedup 1.0000x reference)
"""MoE (top-2 of 8 experts, B=8192, D=2048) on 8 Trainium2 NeuronCores.

Expert-parallel: host computes gate softmax + top-2 routing (float64;
rank margins ~3e-5 so selection matches any f32 platform), dispatches
each token's rows to its experts' cores; core e computes
    y_e = relu(x_e @ W[e].T + b[e]) * gate_scale
as an fp16 tiled matmul (PSUM fp32 accumulation), and the host
scatter-adds the two expert contributions per token.

Schedule: single-phase n-major sweep.  For each 512-wide output column
block n (weights wt[n], 2MB), sweep all m token-tiles with a 16-step
PSUM-accumulated kd loop; the DVE epilogue (bias add, then relu*scale
via one tensor_scalar) drains each bank to fp16 and DMAs it out, so the
post-stream tail is one epilogue (~1.6us) instead of three.

DMA routing exploits that each DGE ring's descriptors execute FIFO
while the two rings race each other for shared HBM: everything
sequencing-critical rides the scalar ring in exact consumption order
(xt0, wt0 odd kd-groups, scale, 8KB bias row, xt1..16, then wt1..3
which land ~49/55/62us, far ahead of their sweeps), and the sync ring
carries only wt0's even kd-groups (doubling the feed rate for the
DMA-paced first m-tile) plus the y-out stream.  Chunks are sized for
>=2KB per-partition DMA lines (1KB-line transfers only sustain
~90-160GB/s).  The cold-clock (HAM 1.2GHz) window warms on real work;
the 8KB bias row is broadcast to 128 partitions by the otherwise-idle
GpSimd engine.  LDWEIGHTS is fully hidden by the PE reorder window
(measured 216ns/MM steady = N/2.4GHz + NX dispatch, the hardware
floor), so the kernel streams at the PE roofline for the 17-tile padded
token count; remaining fixed costs are the ~5.5us engine preamble,
~3us DMA startup lag, and the ~11us end-of-kernel semaphore teardown,
none of which scale with kernel structure.  Measured 258.7us max-core
(257.0 mean) vs the 297.9us session baseline.
"""

import math

import numpy as np

B, D, E, TOP_K = 8192, 2048, 8, 2
N_CORES = 8
P = 128
KD = D // P  # 16 contraction chunks
NT = 4
NSZ = D // NT  # 512 output columns per psum tile

_F16 = np.float16

_nc_cache = {}


def _routing(x, Wg, bg):
    """Gate softmax + top-2 in float64; returns (idx [B,2] int, vals [B,2] f32)."""
    logits = x.astype(np.float64) @ Wg.astype(np.float64).T + bg.astype(np.float64)
    logits -= logits.max(-1, keepdims=True)
    eL = np.exp(logits)
    gate = eL / eL.sum(-1, keepdims=True)
    order = np.argsort(-gate, axis=-1, kind="stable")
    idx = order[:, :TOP_K]
    vals = np.take_along_axis(gate, idx, -1).astype(np.float32)
    return idx, vals


def _build(m_tiles):
    """Build + compile the per-core Bass kernel for C = m_tiles*128 tokens."""
    import concourse.mybir as mybir
    import concourse.tile as tile
    from concourse import bacc

    nc = bacc.Bacc("TRN2", target_bir_lowering=False)
    C = m_tiles * P
    xt = nc.dram_tensor("xt", [P, m_tiles, KD, P], mybir.dt.float16, kind="ExternalInput")
    wt = nc.dram_tensor("wt", [P, NT, KD, NSZ], mybir.dt.float16, kind="ExternalInput")
    bias = nc.dram_tensor("bias", [1, D], mybir.dt.float32, kind="ExternalInput")
    scale = nc.dram_tensor("scale", [P, m_tiles], mybir.dt.float32, kind="ExternalInput")
    y = nc.dram_tensor("y", [C, D], mybir.dt.float16, kind="ExternalOutput")

    with tile.TileContext(nc) as tc:
        with (
            tc.tile_pool(name="wp", bufs=1) as wp,
            tc.tile_pool(name="xp", bufs=1) as xp,
            tc.tile_pool(name="cp", bufs=1) as cp,
            tc.tile_pool(name="op", bufs=8) as op_,
            tc.tile_pool(name="pp", bufs=8, space="PSUM") as pp,
        ):
            # Per-ring descriptors execute FIFO, so each ring's transfer
            # order is its program order — but the two rings race each other
            # for shared HBM (~190GB/s each when both busy).  Everything
            # sequencing-critical therefore rides ONE ring (scalar/Q10) in
            # exact consumption order, while the other ring (sync/Q1)
            # carries only wt[0]'s even kd-groups (to double the feed rate
            # for the DMA-paced first m-tile) and the y-out stream.  Chunks
            # are >=2KB per-partition lines (1KB-line transfers only sustain
            # ~90-160GB/s; 4KB+ reach ~300+GB/s).
            wts = [None] * NT
            wts[0] = wp.tile([P, KD, NSZ], mybir.dt.float16, tag="wt0", name="wt_sb0")
            xts = [None] * m_tiles
            xts[0] = xp.tile([P, KD, P], mybir.dt.float16, tag="xt0", name="xt_sb0")
            xts[1] = xp.tile([P, KD, P], mybir.dt.float16, tag="xt1", name="xt_sb1")
            nc.scalar.dma_start(xts[0][:, 0:8], xt[:, 0, 0:8])
            nc.sync.dma_start(wts[0][:, 0:2], wt[:, 0, 0:2])
            nc.sync.dma_start(xts[1][:, 0:8], xt[:, 1, 0:8])
            nc.sync.dma_start(wts[0][:, 2:4], wt[:, 0, 2:4])
            nc.sync.dma_start(wts[0][:, 8:12], wt[:, 0, 8:12])
            nc.scalar.dma_start(wts[0][:, 4:8], wt[:, 0, 4:8])
            scale_sb = cp.tile([P, m_tiles], mybir.dt.float32, tag="scale", name="scale_sb")
            nc.scalar.dma_start(scale_sb[:], scale[:])
            bias_row = cp.tile([1, D], mybir.dt.float32, tag="bias_row", name="bias_row")
            nc.scalar.dma_start(bias_row[:], bias[:])
            bias_sb = cp.tile([P, D], mybir.dt.float32, tag="bias", name="bias_sb")
            nc.gpsimd.partition_broadcast(bias_sb[:], bias_row[:])
            nc.scalar.dma_start(xts[0][:, 8:16], xt[:, 0, 8:16])
            nc.scalar.dma_start(xts[1][:, 8:16], xt[:, 1, 8:16])
            nc.scalar.dma_start(wts[0][:, 12:16], wt[:, 0, 12:16])
            for m in range(2, m_tiles):
                xts[m] = xp.tile([P, KD, P], mybir.dt.float16, tag=f"xt{m}", name=f"xt_sb{m}")
                nc.scalar.dma_start(xts[m][:], xt[:, m])
            # Remaining weight blocks queue behind the token stream on the
            # scalar ring — FIFO lands them at ~49/55/62us, well before
            # their sweeps start (~77/135/193us).
            for n in range(1, NT):
                wts[n] = wp.tile([P, KD, NSZ], mybir.dt.float16, tag=f"wt{n}", name=f"wt_sb{n}")
                nc.scalar.dma_start(wts[n][:], wt[:, n])

            def epilogue(ps, m, n):
                ot = op_.tile([P, NSZ], mybir.dt.float16, tag="ot", name="ot")
                tmp = op_.tile([P, NSZ], mybir.dt.float32, tag="tmp", name="tmp")
                nc.vector.tensor_tensor(
                    tmp[:], ps[:], bias_sb[:, n * NSZ:(n + 1) * NSZ], mybir.AluOpType.add
                )
                # relu((z+b)*s) with s>=0, via one DVE tensor_scalar (mult, max 0)
                nc.vector.tensor_scalar(
                    ot[:], tmp[:], scale_sb[:, m:m + 1], 0.0,
                    mybir.AluOpType.mult, mybir.AluOpType.max,
                )
                nc.sync.dma_start(y[m * P:(m + 1) * P, n * NSZ:(n + 1) * NSZ], ot[:])

            # First two tiles of the n0 sweep interleave their kd loops:
            # each wt[0] kd-chunk then feeds TWO matmuls, so the PE does 2x
            # the work per delivered weight byte during the ramp, where the
            # DMA fabric is still slow (~74->190GB/s per ring over the first
            # ~5us) and delivery — not compute — paces the sweep.  (A 3-way
            # interleave measured slightly worse: the third tile's 0.5MB at
            # the head of the queue costs more than its coverage adds.)
            pss = [pp.tile([P, NSZ], mybir.dt.float32, tag="ps", name="ps")
                   for _ in range(2)]
            for kd in range(KD):
                for j in range(2):
                    nc.tensor.matmul(pss[j][:], xts[j][:, kd], wts[0][:, kd],
                                     start=(kd == 0), stop=(kd == KD - 1))
            for j in range(2):
                epilogue(pss[j], j, 0)

            for n in range(NT):
                for m in range(2 if n == 0 else 0, m_tiles):
                    ps = pp.tile([P, NSZ], mybir.dt.float32, tag="ps", name="ps")
                    for kd in range(KD):
                        nc.tensor.matmul(
                            ps[:], xts[m][:, kd], wts[n][:, kd],
                            start=(kd == 0), stop=(kd == KD - 1),
                        )
                    epilogue(ps, m, n)

    nc.compile()
    return nc


def _get_nc(m_tiles):
    if m_tiles not in _nc_cache:
        _nc_cache[m_tiles] = _build(m_tiles)
    return _nc_cache[m_tiles]


def _prep_inputs(x, W, b, idx, vals):
    """Per-core input maps: blocked fp16 xT/wT layouts + bias/scale tiles."""
    in_maps = []
    token_lists = []
    counts = []
    for e in range(E):
        tok = np.where((idx == e).any(axis=1))[0]
        token_lists.append(tok)
        counts.append(len(tok))
    c_max = max(counts)
    m_tiles = max(1, math.ceil(c_max / P))
    C = m_tiles * P

    for e in range(E):
        tok = token_lists[e]
        cnt = len(tok)
        Xp = np.zeros((C, D), dtype=_F16)
        Xp[:cnt] = x[tok].astype(_F16)
        xt_np = np.ascontiguousarray(
            Xp.reshape(m_tiles, P, KD, P).transpose(3, 0, 2, 1)
        )
        wt_np = np.ascontiguousarray(
            W[e].astype(_F16).reshape(NT, NSZ, KD, P).transpose(3, 0, 2, 1)
        )
        bias_np = np.ascontiguousarray(b[e].reshape(1, D).astype(np.float32))
        s_tok = np.zeros(C, dtype=np.float32)
        for k in range(TOP_K):
            sel = idx[tok, k] == e
            s_tok[:cnt][sel] = vals[tok[sel], k]
        scale_np = np.ascontiguousarray(s_tok.reshape(m_tiles, P).T)
        in_maps.append({"xt": xt_np, "wt": wt_np, "bias": bias_np, "scale": scale_np})
    return in_maps, token_lists, counts, m_tiles


def kernel(x, W, b, Wg, bg):
    from concourse.bass_utils import run_bass_kernel_spmd

    x = np.asarray(x, dtype=np.float32)
    W = np.asarray(W, dtype=np.float32)
    b = np.asarray(b, dtype=np.float32)
    Wg = np.asarray(Wg, dtype=np.float32)
    bg = np.asarray(bg, dtype=np.float32)

    idx, vals = _routing(x, Wg, bg)
    in_maps, token_lists, counts, m_tiles = _prep_inputs(x, W, b, idx, vals)
    nc = _get_nc(m_tiles)
    res = run_bass_kernel_spmd(nc, in_maps, core_ids=list(range(N_CORES)))

    out = np.zeros((B, D), dtype=np.float32)
    for e in range(E):
        ye = res.results[e]["y"]
        out[token_lists[e]] += ye[:counts[e]].astype(np.float32)
    return out
